# revision 2
# baseline (speedup 1.0000x reference)
"""Bayesian NN Monte-Carlo sampling kernel for 8 TRN2 NeuronCores.

Shards the n_samples axis (S=100 -> 13 per core, 4 padded) across 8 cores.
All math is general (std computed on device from the logvar tensors); host
prep is layout/dtype-only (bf16 cast + reshape/transpose).

Layout trick: features are interleaved mod 4 and contraction rows grouped
p-major on the host, so that
  - we0[s] streams as [112 partitions x 7168B contiguous] lines,
  - we1[s] streams as [128 partitions x 4096B contiguous] lines,
both hitting the large-descriptor DMA regime (~350 GB/s vs ~210 for the
1KB-line strided layout), and
  - each layer's relu output lands exactly in the next layer's contraction
    layout (partition p holds features 4p..4p+3), so no transposes anywhere.

Per core, per sample s (everything bf16 on the PE):
  t_w0 = we0_eps * std0 (one DVE mul), L0 psum[128,256] = W0.T @ x chunks,
  a1 = relu(psum + y0T + b0) where y0T = x@wm0 precomputed once,
  L1 accumulates BOTH eps-weight and wm1 (mean) matmuls into one psum bank
  (mean add moved from DVE to PE), L2 batches all 13 samples' logits into a
  single [64,130] psum bank with the ind-matmul bias trick.
"""

import os
import sys

import numpy as np

if "/opt/trn_rl_repo" not in sys.path:
    sys.path.insert(0, "/opt/trn_rl_repo")

import concourse.bass as bass
from concourse import bacc, mybir, tile
from concourse.bass_utils import run_bass_kernel_spmd

S, B = 100, 64
D0, D1, D2, DO = 784, 512, 512, 10
NCORES = 8
SP = 13           # samples per core; 8*13 = 104, last 4 are wrap padding
P0, T0 = 112, 7   # layer-0 contraction: k = 7*p + t (p-major)
P1, T1 = 128, 4   # layer-1/2 contraction: k = 4*p + t (p-major)
C1 = 4            # feature chunks (features 4*q + c on chunk c, partition q)

F32 = mybir.dt.float32
F32R = mybir.dt.float32r
BF16 = mybir.dt.bfloat16

_CACHE = {}


def _build(mode="bf16"):
    io_dt = BF16
    ts = bass.ts
    AF = mybir.ActivationFunctionType

    nc = bacc.Bacc("TRN2", target_bir_lowering=False, debug=False,
                   num_devices=NCORES)

    def inp(name, shape, dt=io_dt):
        return nc.dram_tensor(name, shape, dt, kind="ExternalInput").ap()

    # p-major / mod-4-interleaved host layouts (see _prep_in_maps)
    xT = inp("xT", [P0, T0 * B])
    wm0 = inp("wm0", [P0, T0 * D1])
    wv0 = inp("wv0", [P0, T0 * D1])
    wm1 = inp("wm1", [P1, T1 * D2])
    wv1 = inp("wv1", [P1, T1 * D2])
    wmlT = inp("wmlT", [P1, T1 * DO])
    wvlT = inp("wvlT", [P1, T1 * DO])
    welT = inp("welT", [P1, SP * T1 * DO])
    we0 = inp("we0", [SP, P0, T0 * D1])
    we1 = inp("we1", [SP, P1, T1 * D2])

    # biases (fp32, small)
    bv0T = inp("bv0T", [P1, C1], F32)
    bm0T = inp("bm0T", [P1, C1], F32)
    be0T = inp("be0T", [P1, C1 * SP], F32)
    bv1T = inp("bv1T", [P1, C1], F32)
    bm1T = inp("bm1T", [P1, C1], F32)
    be1T = inp("be1T", [P1, C1 * SP], F32)
    bvl = inp("bvl", [1, DO])
    bml = inp("bml", [1, DO])
    bel = inp("bel", [SP, DO])
    ones13 = inp("ones13", [1, SP])
    ind = inp("ind", [SP, SP * B])
    out = nc.dram_tensor("out", [B, SP * DO], F32, kind="ExternalOutput").ap()

    with tile.TileContext(nc) as tc:
        with tc.tile_pool(name="const", bufs=1) as const, \
             tc.tile_pool(name="w0e", bufs=3) as w0e, \
             tc.tile_pool(name="w0s", bufs=2) as w0s, \
             tc.tile_pool(name="w1e", bufs=3) as w1e, \
             tc.tile_pool(name="w1s", bufs=2) as w1s, \
             tc.tile_pool(name="wls", bufs=2) as wls, \
             tc.tile_pool(name="acts", bufs=2) as acts, \
             tc.tile_pool(name="bias", bufs=1) as bias, \
             tc.tile_pool(name="ps0", bufs=2, space="PSUM") as ps0, \
             tc.tile_pool(name="ps1", bufs=2, space="PSUM") as ps1, \
             tc.tile_pool(name="ps_b", bufs=1, space="PSUM") as ps_b, \
             tc.tile_pool(name="ps_o", bufs=1, space="PSUM") as ps_o:

            # ---------------- one-time setup ----------------
            # std tensors lead the ring: sample-0 weight prep needs them
            tmp0 = w0e.tile([P0, T0 * D1], io_dt, tag="t_we0")
            nc.sync.dma_start(tmp0[:], wv0[:, :])
            t_std0 = const.tile([P0, T0 * D1], io_dt)
            nc.scalar.activation(t_std0[:], tmp0[:], AF.Exp, scale=0.5)

            tmp1 = w1e.tile([P1, T1 * D2], io_dt, tag="t_we1")
            nc.sync.dma_start(tmp1[:], wv1[:, :])
            t_std1 = const.tile([P1, T1 * D2], io_dt)
            nc.scalar.activation(t_std1[:], tmp1[:], AF.Exp, scale=0.5)

            t_xT = const.tile([P0, T0 * B], io_dt)
            nc.scalar.dma_start(t_xT[:], xT[:, :])

            tmpl = wls.tile([P1, T1 * DO], io_dt, tag="t_wls")
            nc.scalar.dma_start(tmpl[:], wvlT[:, :])
            t_stdl = const.tile([P1, T1 * DO], io_dt)
            nc.scalar.activation(t_stdl[:], tmpl[:], AF.Exp, scale=0.5)
            t_wml = const.tile([P1, T1 * DO], io_dt)
            nc.scalar.dma_start(t_wml[:], wmlT[:, :])
            t_wel = const.tile([P1, SP * T1 * DO], io_dt)
            nc.scalar.dma_start(t_wel[:], welT[:, :])

            # chunk-layout hidden biases: bT[q, c*SP+s] = b_s[4q+c]
            def make_bias_T(bvT, bmT, beT, name):
                vt = bias.tile([P1, C1], F32, tag="vT")
                nc.scalar.dma_start(vt[:], bvT[:, :])
                st = bias.tile([P1, C1], F32, tag="sT")
                nc.scalar.activation(st[:], vt[:], AF.Exp, scale=0.5)
                mt = bias.tile([P1, C1], F32, tag="mT")
                nc.scalar.dma_start(mt[:], bmT[:, :])
                et = bias.tile([P1, C1 * SP], F32, tag="eT")
                nc.scalar.dma_start(et[:], beT[:, :])
                bt = const.tile([P1, C1 * SP], F32, tag=name)
                for c in range(C1):
                    nc.vector.tensor_scalar_mul(
                        bt[:, ts(c, SP)], et[:, ts(c, SP)], st[:, c:c + 1])
                    nc.vector.tensor_scalar_add(
                        bt[:, ts(c, SP)], bt[:, ts(c, SP)], mt[:, c:c + 1])
                return bt

            t_bT0 = make_bias_T(bv0T, bm0T, be0T, "bT0")
            t_bT1 = make_bias_T(bv1T, bm1T, be1T, "bT1")

            # last-layer bias rows [SP, DO]: ones-matmul broadcast
            t_ones13 = const.tile([1, SP], io_dt)
            nc.scalar.dma_start(t_ones13[:], ones13[:, :])

            def bcast(row, D, tag):
                pb = ps_b.tile([SP, D], F32, tag="bb")
                nc.tensor.matmul(pb[:], t_ones13[:], row[:],
                                 start=True, stop=True)
                sbuf = bias.tile([SP, D], io_dt, tag=tag)
                nc.scalar.copy(sbuf[:], pb[:])
                return sbuf

            r = bias.tile([1, DO], io_dt, tag="brow")
            nc.scalar.dma_start(r[:], bvl[:, :])
            sb = bias.tile([1, DO], io_dt, tag="brow2")
            nc.scalar.activation(sb[:], r[:], AF.Exp, scale=0.5)
            sbb = bcast(sb, DO, "bb1")
            mr = bias.tile([1, DO], io_dt, tag="brow3")
            nc.scalar.dma_start(mr[:], bml[:, :])
            mb = bcast(mr, DO, "bb2")
            eb = bias.tile([SP, DO], io_dt, tag="bb3")
            nc.scalar.dma_start(eb[:], bel[:, :])
            ba = bias.tile([SP, DO], io_dt, tag="bb4")
            nc.vector.tensor_mul(ba[:], eb[:], sbb[:])
            t_bl = bias.tile([SP, DO], io_dt, tag="ball")
            nc.vector.tensor_add(t_bl[:], ba[:], mb[:])

            t_ind = const.tile([SP, SP * B], io_dt)
            nc.scalar.dma_start(t_ind[:], ind[:, :])

            # means after the first eps tiles on the sync ring
            t_wm0 = const.tile([P0, T0 * D1], io_dt)
            t_wm1 = const.tile([P1, T1 * D2], io_dt)

            t_out = const.tile([B, SP * DO], F32)

            def mm(psum, lhsT, rhs, start, stop, skip=False):
                nc.tensor.matmul(psum, lhsT, rhs, start=start, stop=stop,
                                 skip_group_check=skip)

            # y0T[q, c*64+b] = (x @ wm0)[4q+c, b], precomputed once
            def make_y0T():
                py0 = ps0.tile([P1, C1 * B], F32, tag="p0")
                for c in range(C1):
                    for t in range(T0):
                        mm(py0[:, ts(c, B)],
                           t_wm0[:, t * D1 + c * P1: t * D1 + (c + 1) * P1],
                           t_xT[:, ts(t, B)],
                           start=(t == 0), stop=(t == T0 - 1))
                y0 = const.tile([P1, C1 * B], F32R)
                nc.scalar.copy(y0[:], py0[:])
                return y0

            # ---------------- per-sample pipeline ----------------
            def weight_prep(s, first=False):
                t_we0 = w0e.tile([P0, T0 * D1], io_dt, tag="t_we0")
                nc.sync.dma_start(t_we0[:], we0[s])
                if first:
                    nc.sync.dma_start(t_wm0[:], wm0[:, :])
                t_w0 = w0s.tile([P0, T0 * D1], io_dt)
                nc.vector.tensor_mul(t_w0[:], t_we0[:], t_std0[:])

                t_we1 = w1e.tile([P1, T1 * D2], io_dt, tag="t_we1")
                nc.sync.dma_start(t_we1[:], we1[s])
                if first:
                    nc.sync.dma_start(t_wm1[:], wm1[:, :])
                t_w1 = w1s.tile([P1, T1 * D2], io_dt)
                nc.vector.tensor_mul(t_w1[:], t_we1[:], t_std1[:])

                t_wl = wls.tile([P1, T1 * DO], io_dt, tag="t_wls")
                nc.vector.tensor_mul(t_wl[:], t_wel[:, ts(s, T1 * DO)],
                                     t_stdl[:])
                t_wlf = wls.tile([P1, T1 * DO], io_dt, tag="t_wlf")
                nc.vector.tensor_add(t_wlf[:], t_wl[:], t_wml[:])
                return t_w0, t_w1, t_wlf

            def compute(s, t_w0, t_w1, t_wlf, t_y0T, po):
                # layer 0: all 4 chunks into one psum bank
                p0 = ps0.tile([P1, C1 * B], F32, tag="p0")
                for c in range(C1):
                    for t in range(T0):
                        mm(p0[:, ts(c, B)],
                           t_w0[:, t * D1 + c * P1: t * D1 + (c + 1) * P1],
                           t_xT[:, ts(t, B)],
                           start=(t == 0), stop=(t == T0 - 1))
                a1p = acts.tile([P1, C1 * B], F32R, tag="a1p")
                nc.vector.tensor_add(a1p[:], p0[:], t_y0T[:])
                a1T = acts.tile([P1, C1 * B], io_dt, tag="a1T")
                for c in range(C1):
                    nc.scalar.activation(
                        a1T[:, ts(c, B)], a1p[:, ts(c, B)], AF.Relu,
                        bias=t_bT0[:, c * SP + s: c * SP + s + 1])

                # layer 1: eps-weight + mean matmuls into one psum bank
                p1 = ps1.tile([P1, C1 * B], F32, tag="p1")
                a2T = acts.tile([P1, C1 * B], io_dt, tag="a2T")
                for c in range(C1):
                    for t in range(T1):
                        mm(p1[:, ts(c, B)],
                           t_w1[:, t * D2 + c * P1: t * D2 + (c + 1) * P1],
                           a1T[:, ts(t, B)],
                           start=(t == 0), stop=False)
                    for t in range(T1):
                        mm(p1[:, ts(c, B)],
                           t_wm1[:, t * D2 + c * P1: t * D2 + (c + 1) * P1],
                           a1T[:, ts(t, B)],
                           start=False, stop=(t == T1 - 1))
                    nc.scalar.activation(
                        a2T[:, ts(c, B)], p1[:, ts(c, B)], AF.Relu,
                        bias=t_bT1[:, c * SP + s: c * SP + s + 1])

                # output layer: all samples share one [64, SP*DO] psum bank
                for t in range(T1):
                    mm(po[:, ts(s, DO)], a2T[:, ts(t, B)],
                       t_wlf[:, ts(t, DO)], start=(t == 0), stop=False)
                mm(po[:, ts(s, DO)], t_ind[:, ts(s, B)], t_bl[:],
                   start=False, stop=True)

            po = ps_o.tile([B, SP * DO], F32, tag="out")
            prep = weight_prep(0, first=True)
            t_y0T = make_y0T()
            for s in range(SP):
                compute(s, *prep, t_y0T, po)
                prep = weight_prep(s + 1) if s + 1 < SP else None

            nc.scalar.copy(t_out[:], po[:])
            nc.sync.dma_start(out[:, :], t_out[:])

    nc.compile()
    return nc


def _get_nc(mode="bf16"):
    if "nc" not in _CACHE:
        _CACHE["nc"] = _build()
    return _CACHE["nc"]


def _prep_in_maps(inputs, mode="bf16"):
    import ml_dtypes
    np_dt = ml_dtypes.bfloat16

    def cvt(a):
        return np.ascontiguousarray(a).astype(np_dt, copy=False)

    def f32(a):
        return np.ascontiguousarray(np.asarray(a, np.float32))

    x = np.asarray(inputs["inputs"], np.float32)
    we0 = np.asarray(inputs["we0"], np.float32)
    we1 = np.asarray(inputs["we1"], np.float32)
    wel = np.asarray(inputs["wel"], np.float32)
    be0 = np.asarray(inputs["be0"], np.float32).reshape(S, D1)
    be1 = np.asarray(inputs["be1"], np.float32).reshape(S, D2)
    bel = np.asarray(inputs["bel"], np.float32).reshape(S, DO)

    # p-major rows + mod-4 interleaved feature columns:
    #   out[p, (t, c, q)] = M[T*p + t, 4*q + c]
    def pm0(M):  # [784, 512] -> [112, 7*512]
        return M.reshape(P0, T0, P1, C1).transpose(0, 1, 3, 2) \
                .reshape(P0, T0 * D1)

    def pm1(M):  # [512, 512] -> [128, 4*512]
        return M.reshape(P1, T1, P1, C1).transpose(0, 1, 3, 2) \
                .reshape(P1, T1 * D2)

    def pml(M):  # [512, 10] -> [128, 4*10] (row permutation only)
        return M.reshape(P1, T1, DO).reshape(P1, T1 * DO)

    # xT[p, t*B+b] = x[b, 7p+t]
    xTpm = x.T.reshape(P0, T0, B).reshape(P0, T0 * B)

    def bias_T(b):  # [SP, D] -> [128, C1*SP] with [q, c*SP+s] = b[s, 4q+c]
        return np.ascontiguousarray(
            b.reshape(SP, P1, C1).transpose(1, 2, 0).reshape(P1, C1 * SP))

    def bias_cq(v):  # [D] -> [128, C1] with [q, c] = v[4q+c]
        return np.ascontiguousarray(np.asarray(v, np.float32)
                                    .reshape(P1, C1))

    shared = {
        "xT": cvt(xTpm),
        "wm0": cvt(pm0(np.asarray(inputs["wm0"], np.float32))),
        "wv0": cvt(pm0(np.asarray(inputs["wv0"], np.float32))),
        "wm1": cvt(pm1(np.asarray(inputs["wm1"], np.float32))),
        "wv1": cvt(pm1(np.asarray(inputs["wv1"], np.float32))),
        "wmlT": cvt(pml(np.asarray(inputs["wml"], np.float32))),
        "wvlT": cvt(pml(np.asarray(inputs["wvl"], np.float32))),
        "bv0T": bias_cq(inputs["bv0"]),
        "bm0T": bias_cq(inputs["bm0"]),
        "bv1T": bias_cq(inputs["bv1"]),
        "bm1T": bias_cq(inputs["bm1"]),
        "bvl": cvt(np.asarray(inputs["bvl"], np.float32).reshape(1, DO)),
        "bml": cvt(np.asarray(inputs["bml"], np.float32).reshape(1, DO)),
        "ones13": cvt(np.ones((1, SP), np.float32)),
        "ind": cvt(np.repeat(np.eye(SP, dtype=np.float32), B, axis=1)),
    }

    def shard(a, k):
        lo = k * SP
        hi = lo + SP
        if hi <= S:
            return a[lo:hi]
        return np.concatenate([a[lo:S], a[: hi - S]], axis=0)

    in_maps = []
    for k in range(NCORES):
        welk = shard(wel, k)  # [SP, 512, 10]
        in_maps.append(dict(
            shared,
            we0=cvt(np.stack([pm0(m) for m in shard(we0, k)])),
            we1=cvt(np.stack([pm1(m) for m in shard(we1, k)])),
            welT=cvt(np.stack([pml(m) for m in welk], axis=1)
                     .reshape(P1, SP * T1 * DO)),
            be0T=bias_T(shard(be0, k)),
            be1T=bias_T(shard(be1, k)),
            bel=cvt(shard(bel, k)),
        ))
    return in_maps


def _run(inputs, mode="bf16", trace=False):
    nc = _get_nc(mode)
    in_maps = _prep_in_maps(inputs, mode)
    res = run_bass_kernel_spmd(nc, in_maps, core_ids=list(range(NCORES)),
                               trace=trace)
    outs = []
    for k in range(NCORES):
        o = np.asarray(res.results[k]["out"], np.float32)  # [64, 130]
        outs.append(o.reshape(B, SP, DO).transpose(1, 0, 2))
    full = np.concatenate(outs, axis=0)[:S]  # [100, 64, 10]
    return full, res


def kernel(**inputs):
    out, _ = _run(inputs)
    return out
